# revision 18
# baseline (speedup 1.0000x reference)
"""Cross-attention Trainium2 kernel, 8 NeuronCores, no device collectives.

Head-sharded: core j computes head j for the full sequence and both
batches, emitting the partial output Wout[:, head_j] @ attn_j for the
whole [B, C, S] output. The host converts inputs to bf16, uploads the
FULL x/c to every core (upload is host-side, not kernel time), and sums
the 8 bf16 partial outputs in f32, adding bout.

Compute is bf16 with f32 PSUM accumulation. DH = 160 = 128 + 32; all
32-row tail matmuls contract on PE rows 0:32 (no tile packing — it
gives no concurrency under this toolchain). The PE stream is kept
dense and wait-light: K/V projections for both batches first (k-tail
chains t-outer across 4 PSUM banks so the stationary is shared), then
one loop over 16 (batch, q-block) pairs. Scores run in groups of 4 key
chunks; the group's attn matmuls are batched (4x pa1 then 4x pa2) one
group behind so a single vector-clock wait covers all four exps. Each
pair's Q-projection is issued during the previous pair's output phase,
and softmax normalization is applied AFTER the output projection
(Wout @ (attn diag(1/den)) == (Wout @ attn) diag(1/den)), keeping the
reciprocal chain entirely off the PE.
"""

import sys

sys.path.insert(0, "/opt/trn_rl_repo")

import ml_dtypes
import numpy as np

import concourse.bacc as bacc
import concourse.tile as tile
from concourse import mybir
from concourse.bass_utils import run_bass_kernel_spmd

HEADS = 8
DH = 160
C = 1280
B = 2
S = 4096
SH = 512  # q-block size
NJ = S // SH  # 8 q blocks
CT = C // 128  # contraction tiles
KC = S // 128  # key chunks
SCALE = DH ** -0.5
BF16 = ml_dtypes.bfloat16

_cache = {}


def _build():
    if "nc" in _cache:
        return _cache["nc"]
    f32 = mybir.dt.float32
    bf16 = mybir.dt.bfloat16
    f32r = mybir.dt.float32r

    nc = bacc.Bacc("TRN2", target_bir_lowering=False, debug=False,
                   num_devices=HEADS)
    d_x = nc.dram_tensor("x", [B, C, S], bf16, kind="ExternalInput").ap()
    d_c = nc.dram_tensor("c", [B, C, S], bf16, kind="ExternalInput").ap()
    # w columns: 0:128 WqT head-dims 0:128 | 128:256 WkT 0:128 |
    #            256:288 WqT 128:160 | 288:320 WkT 128:160 | 320:480 WvT
    d_w = nc.dram_tensor("w", [C, 480], bf16, kind="ExternalInput").ap()
    d_wo = nc.dram_tensor("wo", [DH, C], bf16, kind="ExternalInput").ap()
    d_msk = nc.dram_tensor("msk", [B, S], f32, kind="ExternalInput").ap()
    d_out = nc.dram_tensor("out", [B, C, S], bf16, kind="ExternalOutput").ap()

    with tile.TileContext(nc) as tc:
        with (
            tc.tile_pool(name="wp", bufs=1) as wp,
            tc.tile_pool(name="big", bufs=1) as big,
            tc.tile_pool(name="stream", bufs=4) as stream,
            tc.tile_pool(name="smal", bufs=2) as smal,
            tc.tile_pool(name="expp", bufs=14) as expp,
            tc.tile_pool(name="outp", bufs=3) as outp,
            tc.tile_pool(name="psS", bufs=4, space="PSUM") as psS,
            tc.tile_pool(name="psa", bufs=1, space="PSUM") as psa,
            tc.tile_pool(name="pso", bufs=2, space="PSUM") as pso,
        ):
            # ---- weights / mask / ones ----
            wqkv = wp.tile([128, CT, 480], bf16, tag="wqkv")
            w_r = d_w.rearrange("(t p) d -> p t d", p=128)
            # k-section first: it gates the first kA matmul at kernel start
            nc.gpsimd.dma_start(out=wqkv[:, :, 128:320], in_=w_r[:, :, 128:320])
            nc.gpsimd.dma_start(out=wqkv[:, :, 320:480], in_=w_r[:, :, 320:480])
            nc.gpsimd.dma_start(out=wqkv[:, :, 0:128], in_=w_r[:, :, 0:128])
            woA = wp.tile([128, C], bf16, tag="woA")
            nc.gpsimd.dma_start(out=woA, in_=d_wo[0:128, :])
            woB = wp.tile([32, C], bf16, tag="woB")
            nc.gpsimd.dma_start(out=woB, in_=d_wo[128:160, :])
            msk = wp.tile([128, B, KC], f32, tag="msk")
            nc.gpsimd.dma_start(out=msk,
                                in_=d_msk.rearrange("b (t p) -> p b t", p=128))
            ones_f = wp.tile([1, 128], f32, tag="onesf")
            nc.vector.memset(ones_f, 1.0)
            ones_col = wp.tile([1, 128], f32r, tag="ones")
            with nc.allow_low_precision(reason="f32r rounding for PE broadcast"):
                nc.vector.tensor_copy(out=ones_col, in_=ones_f)

            kA = {}
            kB = {}
            vT = {}
            for b in range(B):
                kA[b] = big.tile([128, S], bf16, tag=f"kA{b}", name=f"kA{b}")
                # k head-dims 128:160 per key chunk, on partitions 0:32
                kB[b] = big.tile([32, KC, 128], bf16, tag=f"kB{b}",
                                 name=f"kB{b}")
                vT[b] = big.tile([128, KC, DH + 1], bf16, tag=f"vT{b}",
                                 name=f"vT{b}")

            def phase1(b):
                # K/V projections over the full sequence for this head.
                c_r = d_c[b].rearrange("(t p) s -> p t s", p=128)
                for j in range(NJ):
                    sl = slice(SH * j, SH * j + SH)
                    ct = stream.tile([128, CT, SH], bf16, tag="ct")
                    if b == 0 and j == 0:
                        # two DMAs so the first contraction chunks land early
                        nc.sync.dma_start(out=ct[:, 0:2], in_=c_r[:, 0:2, sl])
                        nc.sync.dma_start(out=ct[:, 2:CT], in_=c_r[:, 2:CT, sl])
                    else:
                        nc.sync.dma_start(out=ct, in_=c_r[:, :, sl])
                    pk = psS.tile([128, SH], f32, tag="ps")
                    for t in range(CT):
                        nc.tensor.matmul(out=pk, lhsT=wqkv[:, t, 128:256],
                                         rhs=ct[:, t, :],
                                         start=(t == 0), stop=(t == CT - 1))
                    nc.scalar.copy(out=kA[b][:, sl], in_=pk)
                    # k-tail: 4 chains t-outer across 4 banks so the
                    # stationary (wk tail chunk) is loaded once per t.
                    pkbs = [psS.tile([32, 128], f32, tag="ps",
                                     name=f"pkb{u}")
                            for u in range(4)]
                    for t in range(CT):
                        for u in range(4):
                            nc.tensor.matmul(
                                out=pkbs[u],
                                lhsT=wqkv[:, t, 288:320],
                                rhs=ct[:, t, 128 * u:128 * u + 128],
                                start=(t == 0), stop=(t == CT - 1))
                    for u in range(4):
                        nc.vector.tensor_copy(out=kB[b][:, 4 * j + u, :],
                                              in_=pkbs[u])
                    for u in range(4):
                        kc = 4 * j + u
                        msl = slice(128 * u, 128 * u + 128)
                        pv = psS.tile([128, DH], f32, tag="ps")
                        for t in range(CT):
                            nc.tensor.matmul(out=pv, lhsT=ct[:, t, msl],
                                             rhs=wqkv[:, t, 320:480],
                                             start=(t == 0), stop=(t == CT - 1))
                        nc.vector.tensor_copy(out=vT[b][:, kc, 0:DH], in_=pv)
                nc.vector.memset(vT[b][:, :, DH:DH + 1], 1.0)

            x_r = [d_x[b].rearrange("(t p) s -> p t s", p=128)
                   for b in range(B)]
            out_r = [d_out[b].rearrange("(t p) s -> p t s", p=128)
                     for b in range(B)]

            def qproj(bj, mid=None):
                # Q projection for pair bj; issued one pair ahead so the PE
                # stays busy through the attn->outproj boundary. `mid` is
                # called a few matmuls in (used to slot the denominator
                # broadcast once its ACT copy has landed).
                b, j = divmod(bj, NJ)
                sl = slice(SH * j, SH * j + SH)
                qa = smal.tile([128, SH], bf16, tag="qa")
                qb = smal.tile([32, SH], bf16, tag="qb")
                ht = stream.tile([128, CT, SH], bf16, tag="ht")
                nc.sync.dma_start(out=ht, in_=x_r[b][:, :, sl])
                pq = psS.tile([128, SH], f32, tag="ps")
                for t in range(CT):
                    nc.tensor.matmul(out=pq, lhsT=wqkv[:, t, 0:128],
                                     rhs=ht[:, t, :],
                                     start=(t == 0), stop=(t == CT - 1))
                    if t == 2 and mid is not None:
                        mid()
                nc.scalar.copy(out=qa, in_=pq)
                pqb = psS.tile([32, SH], f32, tag="ps")
                for t in range(CT):
                    nc.tensor.matmul(out=pqb, lhsT=wqkv[:, t, 256:288],
                                     rhs=ht[:, t, :],
                                     start=(t == 0), stop=(t == CT - 1))
                nc.scalar.copy(out=qb, in_=pqb)
                return qa, qb

            phase1(0)
            cur = qproj(0)
            phase1(1)

            for bj in range(B * NJ):
                b, j = divmod(bj, NJ)
                sl = slice(SH * j, SH * j + SH)
                qa, qb = cur
                pa1 = psa.tile([128, SH], f32, tag="pa1")
                pa2 = psa.tile([33, SH], f32, tag="pa2")

                # Scores in groups of 4 chunks; the group's exps run on ACT
                # while the PE does the next group's scores; attn for group
                # g-1 is batched (4x pa1, then 4x pa2) after scores of g so
                # one wait covers all four exps.
                ets = {}

                def emit_scores(g):
                    pts = []
                    for u in range(4):
                        kc = 4 * g + u
                        ksl = slice(128 * kc, 128 * kc + 128)
                        ps_t = psS.tile([128, SH], f32, tag="ps")
                        nc.tensor.matmul(out=ps_t, lhsT=kA[b][:, ksl],
                                         rhs=qa, start=True, stop=False)
                        pts.append(ps_t)
                    for u in range(4):
                        kc = 4 * g + u
                        nc.tensor.matmul(out=pts[u], lhsT=kB[b][:, kc, :],
                                         rhs=qb, start=False, stop=True)
                    return pts

                def emit_exps(g, pts):
                    for u in range(4):
                        kc = 4 * g + u
                        et = expp.tile([128, SH], bf16, tag="et")
                        nc.scalar.activation(
                            out=et, in_=pts[u],
                            func=mybir.ActivationFunctionType.Exp,
                            bias=msk[:, b, kc:kc + 1], scale=SCALE)
                        ets[kc] = et

                def emit_attn(g):
                    # batched per bank: one vector-clock wait covers all exps
                    for u in range(4):
                        kc = 4 * g + u
                        nc.tensor.matmul(out=pa1, lhsT=vT[b][:, kc, 0:128],
                                         rhs=ets[kc],
                                         start=(kc == 0), stop=(kc == KC - 1))
                    for u in range(4):
                        kc = 4 * g + u
                        nc.tensor.matmul(out=pa2,
                                         lhsT=vT[b][:, kc, 128:DH + 1],
                                         rhs=ets.pop(kc),
                                         start=(kc == 0), stop=(kc == KC - 1))

                for g in range(NJ):
                    pts = emit_scores(g)
                    if g > 0:
                        emit_attn(g - 1)
                    emit_exps(g, pts)
                emit_attn(NJ - 1)

                # denominator row to SBUF on ACT (cheap), then PE-broadcast,
                # then reciprocal with all 128 DVE lanes; normalization is
                # applied after the output projection.
                den = smal.tile([1, SH], f32r, tag="den")
                with nc.allow_low_precision(reason="f32r for PE broadcast"):
                    nc.scalar.copy(out=den, in_=pa2[32:33, :])

                pb_box = []

                def mid():
                    pb = psS.tile([128, SH], f32, tag="ps")
                    nc.tensor.matmul(out=pb, lhsT=ones_col[:], rhs=den[:],
                                     start=True, stop=True)
                    pb_box.append(pb)

                if bj + 1 < B * NJ:
                    cur = qproj(bj + 1, mid=mid)
                else:
                    mid()
                pb = pb_box[0]

                a1 = smal.tile([128, SH], bf16, tag="a1")
                nc.vector.tensor_copy(out=a1, in_=pa1)
                a2 = smal.tile([32, SH], bf16, tag="a2")
                nc.vector.tensor_copy(out=a2, in_=pa2[0:32, :])
                bc = smal.tile([128, SH], f32, tag="bc")
                nc.vector.reciprocal(out=bc, in_=pb)

                last = bj == B * NJ - 1
                for oc in range(CT):
                    osl = slice(128 * oc, 128 * oc + 128)
                    if last:
                        pos = psS.tile([128, SH], f32, tag="ps", name="posL")
                    else:
                        pos = pso.tile([128, SH], f32, tag="po")
                    nc.tensor.matmul(out=pos, lhsT=woA[:, osl], rhs=a1[:],
                                     start=True, stop=False)
                    nc.tensor.matmul(out=pos, lhsT=woB[:, osl], rhs=a2[:],
                                     start=False, stop=True)
                    ot = outp.tile([128, SH], bf16, tag="ot")
                    nc.vector.tensor_mul(ot[:], pos[:], bc[:])
                    nc.sync.dma_start(out=out_r[b][:, oc, sl], in_=ot)

    nc.compile()
    _cache["nc"] = nc
    return nc


def _prep_inputs(hidden_states, context, mask, Wq, Wk, Wv, Wout):
    x = np.asarray(hidden_states, dtype=np.float32)[:, :, 0, :].astype(BF16)
    c = np.asarray(context, dtype=np.float32)[:, :, 0, :].astype(BF16)
    msk = np.ascontiguousarray(np.asarray(mask, dtype=np.float32)[:, :, 0, 0])
    Wq = np.asarray(Wq, dtype=np.float32)
    Wk = np.asarray(Wk, dtype=np.float32)
    Wv = np.asarray(Wv, dtype=np.float32)
    Wout = np.asarray(Wout, dtype=np.float32)
    ins = []
    for j in range(HEADS):
        rows = slice(DH * j, DH * j + DH)
        wq, wk, wv = Wq[rows], Wk[rows], Wv[rows]
        w = np.empty((C, 480), np.float32)
        w[:, 0:128] = wq[0:128].T
        w[:, 128:256] = wk[0:128].T
        w[:, 256:288] = wq[128:160].T
        w[:, 288:320] = wk[128:160].T
        w[:, 320:480] = wv.T
        ins.append({
            "x": x,
            "c": c,
            "w": w.astype(BF16),
            "wo": np.ascontiguousarray(Wout[:, rows].T).astype(BF16),
            "msk": msk,
        })
    return ins


def kernel(hidden_states, context, mask, Wq, Wk, Wv, Wout, bout):
    nc = _build()
    ins = _prep_inputs(hidden_states, context, mask, Wq, Wk, Wv, Wout)
    res = run_bass_kernel_spmd(nc, ins, core_ids=list(range(HEADS)))
    full = np.zeros((B, C, S), np.float32)
    for j in range(HEADS):
        full += np.asarray(res.results[j]["out"], dtype=np.float32)
    full = full + np.asarray(bout, dtype=np.float32)[None, :, None]
    return full[:, :, None, :].astype(np.float32)
